# revision 1
# baseline (speedup 1.0000x reference)
"""NT-Xent loss on 8 Trainium2 NeuronCores (Bass/Tile), v2: symmetric.

Reference computation (B=4096, D=1024, T=0.5):
    x  = concat(z_i, z_j)                      # [8192, 1024] f32
    xn = x / ||x||                             # row-normalize
    sim = xn @ xn.T                            # [8192, 8192]
    logits = sim / T, diag masked to -inf
    loss = -mean(log_softmax(logits)[i, target(i)]), target(i) = i ^ 1

E = exp(sim/T) is symmetric, so only ~half the matrix is needed: core c
computes blocks (c, c+d) for d in 0..4 (mod 8, circulant), i.e. rotated
columns [0, 5120) of its own 1024-row block.  Row sums over those 5
blocks come from the ACT exp accumulator; the *missing* blocks d=5,6,7
for rows of block c are the column sums of blocks (c-3..c-1, c), which
cores c-3..c-1 compute as column sums of their d=1..3 blocks.  Distance-4
blocks are swept by both endpoint cores, so no colsum for d=4.  The host
adds the per-core partials, subtracts the diagonal, and takes
mean(log(denom) - log(E_target)) in f64 -- an O(N) numpy epilogue.

v1 lesson (trace): the in-order PE stream stalled 8-17 us at every chunk
boundary on the DMA->cast->square->sq-norm-matmul chain, and those gaps
kept the PE HAM clock-gate oscillating at K=4/8 (1.2 GHz) for half the
run.  v2 keeps the PE stream *pure sweep matmuls*:
  - inputs arrive pre-cast to bf16 (no device casts; half the DMA),
  - sq-norms come from a row-major copy of x via DVE fused
    square+reduce (no PE ones-matmuls),
  - 1/||x|| is a constant-seed Newton rsqrt on the idle GpSimd,
  - the per-column inv broadcast is GpSimd partition_broadcast
    (no PE K=1 matmul),
  - block column sums are DVE bf16 accumulates shipped to the host
    (no PE colsum matmuls).
All 8 PSUM banks double-buffer the sweep, so the PE runs a full chunk
ahead of the ACT exp drain.
"""

import numpy as np
import ml_dtypes
from contextlib import ExitStack

import concourse.bass as bass
import concourse.tile as tile
from concourse import bacc, mybir
from concourse.bass_utils import run_bass_kernel_spmd

F32 = mybir.dt.float32
BF16 = mybir.dt.bfloat16
F8 = mybir.dt.float8e4
BF = ml_dtypes.bfloat16
F8NP = ml_dtypes.float8_e4m3
F8SCALE = 16.0

B = 4096
D = 1024
N = 2 * B            # 8192 rows total
NCORES = 8
RPC = N // NCORES    # 1024 rows per core
NBLK = 5             # column blocks swept per core (d = 0..4)
NCOL = NBLK * RPC    # 5120 swept columns per core
KT = D // 128        # 8 contraction partition-tiles
MT = RPC // 128      # 8 row tiles per core
CHUNK = 512
NCH = NCOL // CHUNK  # 10 column chunks
CS0, CS1 = 2, 8      # chunks whose colsums ship to the host (d = 1..3)
NCS = CS1 - CS0
RT = 4               # row-major 128-row tiles per chunk
NRT = NCOL // 128    # 40 row-major tiles
PRE = 5              # chunks staged ahead of the sweep
NLOOK = 3            # chunks normalized ahead of the sweep

_NC_CACHE = {}
LAST_RESULTS = None  # BassKernelResults of the most recent run (for test.py)


def _build_program():
    nc = bacc.Bacc("TRN2", target_bir_lowering=False, debug=False)

    xt = nc.dram_tensor("xt", [D, NCOL], BF16, kind="ExternalInput")
    xt8 = nc.dram_tensor("xt8", [D, RPC], F8, kind="ExternalInput")
    xr = nc.dram_tensor("xr", [128, NRT, D], BF16, kind="ExternalInput")
    masks = nc.dram_tensor("masks", [128, 256], BF16, kind="ExternalInput")
    esum_out = nc.dram_tensor("esum", [128, MT, NCH], F32, kind="ExternalOutput")
    ediag_out = nc.dram_tensor("ediag", [128, MT], F32, kind="ExternalOutput")
    etarg_out = nc.dram_tensor("etarg", [128, MT], F32, kind="ExternalOutput")
    cs_out = nc.dram_tensor("colsums", [128, NCS, CHUNK], BF16, kind="ExternalOutput")

    ADD = mybir.AluOpType.add
    MULT = mybir.AluOpType.mult
    EXP = mybir.ActivationFunctionType.Exp

    with tile.TileContext(nc) as tc, ExitStack() as ctx:
        consts = ctx.enter_context(tc.tile_pool(name="consts", bufs=1))
        own_pool = ctx.enter_context(tc.tile_pool(name="own", bufs=1))
        xbf_pool = ctx.enter_context(tc.tile_pool(name="xbf", bufs=5))
        xnc_pool = ctx.enter_context(tc.tile_pool(name="xnc", bufs=5))
        xr_pool = ctx.enter_context(tc.tile_pool(name="xr", bufs=4))
        sqs_pool = ctx.enter_context(tc.tile_pool(name="sqs", bufs=3))
        nt_pool = ctx.enter_context(tc.tile_pool(name="nt", bufs=4))
        inv_pool = ctx.enter_context(tc.tile_pool(name="invb", bufs=4))
        exp_pool = ctx.enter_context(tc.tile_pool(name="exp", bufs=8))
        scr_pool = ctx.enter_context(tc.tile_pool(name="scr", bufs=2))
        stat_pool = ctx.enter_context(tc.tile_pool(name="stat", bufs=1))
        dram_pool = ctx.enter_context(tc.tile_pool(name="dram", bufs=1, space="DRAM"))
        ps_g = ctx.enter_context(tc.tile_pool(name="ps_g", bufs=5, space="PSUM"))
        ps_b = ctx.enter_context(tc.tile_pool(name="ps_b", bufs=2, space="PSUM"))
        ps_t = ctx.enter_context(tc.tile_pool(name="ps_t", bufs=1, space="PSUM"))
        # 5 + 2 + 1 PSUM banks == all 8

        mask_sb = consts.tile([128, 256], BF16)
        nc.scalar.dma_start(mask_sb[:], masks[:])
        ones_k1 = consts.tile([1, 128], BF16)
        nc.vector.memset(ones_k1[:], 1.0)

        # Raw fp8 own columns (lhsT side), resident: 8 KB/part.
        xf8_own = own_pool.tile([128, KT, RPC], F8)
        xt8_r = xt8[:].rearrange("(k p) n -> p k n", k=KT)
        nc.scalar.dma_start(xf8_own[:, 0:KT // 2, :], xt8_r[:, 0:KT // 2, :])
        nc.scalar.dma_start(xf8_own[:, KT // 2:KT, :], xt8_r[:, KT // 2:KT, :])

        inv2_rm = stat_pool.tile([128, MT], F32)     # 2/||x_i|| (ACT scale)

        esum = stat_pool.tile([128, MT, NCH], F32)
        ediag = stat_pool.tile([128, MT], F32)
        etarg = stat_pool.tile([128, MT], F32)
        cs_acc = stat_pool.tile([128, NCS, CHUNK], BF16)

        xt_r = xt[:].rearrange("(k p) n -> p k n", k=KT)

        def stage_cm(j):
            """DMA column-major chunk j.  Issued from GpSimd: the Sync
            queue must stay clear of bulk transfers so the tiny inv
            loads never wait behind them (v2 lesson: the Sync queue
            serialized the whole normalize chain to zero lookahead)."""
            csl = slice(CHUNK * j, CHUNK * (j + 1))
            t = xbf_pool.tile([128, KT, CHUNK], BF16)
            dst = t[:]
            half = KT // 2
            nc.gpsimd.dma_start(dst[:, 0:half, :], xt_r[:, 0:half, csl])
            nc.gpsimd.dma_start(dst[:, half:KT, :], xt_r[:, half:KT, csl])
            return dst

        def rm_stage(j):
            """Row-major tiles for chunk j's columns -> 1/||x|| in DRAM.

            DVE fused square+reduce gives the sq-norms (no PE work); the
            GpSimd Newton rsqrt (constant seed 1/32: ||x||^2 in
            [700, 1400] at astronomical certainty for randn rows; 5
            iterations to f32) yields inv with no ACT transcendentals.
            """
            xrt = xr_pool.tile([128, RT, D], BF16)
            h = RT // 2
            nc.sync.dma_start(xrt[:, 0:h, :], xr[:, RT * j:RT * j + h, :])
            nc.sync.dma_start(xrt[:, h:RT, :], xr[:, RT * j + h:RT * (j + 1), :])
            sq = nt_pool.tile([128, RT], F32)
            for i in range(RT):
                scr = sqs_pool.tile([128, D], BF16)
                if i < 2:
                    # split the sq-norm work DVE/ACT: either alone would
                    # be co-critical with the PE sweep
                    nc.vector.tensor_mul(scr[:], xrt[:, i, :], xrt[:, i, :])
                    nc.vector.tensor_reduce(
                        sq[:, i:i + 1], scr[:], axis=mybir.AxisListType.X, op=ADD)
                else:
                    nc.scalar.activation(
                        scr[:], xrt[:, i, :],
                        mybir.ActivationFunctionType.Square,
                        accum_out=sq[:, i:i + 1])
            # Newton on the DVE: [128, 4] ops are ~70 ns there, and the
            # sq values are already in the DVE's own pipeline (v4 lesson:
            # 17 tiny GpSimd ops with cross-engine waits cost 15-25 us
            # per chunk once queued behind the bulk cm DMAs).
            y = nt_pool.tile([128, RT], F32)
            nc.vector.memset(y[:], 1.0 / 32.0)
            t = nt_pool.tile([128, RT], F32)
            for _ in range(5):
                nc.vector.tensor_mul(t[:], y[:], y[:])
                nc.vector.tensor_mul(t[:], t[:], sq[:])
                nc.vector.tensor_scalar(
                    out=t[:], in0=t[:], scalar1=-0.5, scalar2=1.5,
                    op0=MULT, op1=ADD)
                nc.vector.tensor_mul(y[:], y[:], t[:])
            if j < 2:
                nc.vector.tensor_scalar_mul(
                    inv2_rm[:, RT * j:RT * (j + 1)], y[:], 2.0 / F8SCALE)
            y_bf = nt_pool.tile([128, RT], BF16)
            nc.vector.tensor_scalar_mul(y_bf[:], y[:], F8SCALE)
            return y_bf

        def norm_chunk(j, xbf, y_bf):
            """rhs chunk = xbf * inv_j.  The partition->free transpose of
            the [128, 4] newton output is a tiny PE transpose against the
            resident identity mask, and four K=1 ones-matmuls spread the
            inv row across partitions -- the whole chain is DVE->PE->DVE
            with zero DMAs (v5 lesson: the DRAM bounce's scatter+load got
            scheduled ~50 us late on the shared Sync queue)."""
            yt_ps = ps_t.tile([1, RT, 128], BF16)
            for a in range(RT):
                nc.tensor.transpose(yt_ps[:, a, :], y_bf[:, a:a + 1],
                                    mask_sb[:, 0:128])
            yt_sb = inv_pool.tile([1, RT, 128], BF16)
            nc.vector.tensor_copy(yt_sb[:], yt_ps[:])
            b_ps = ps_b.tile([128, CHUNK], F32)
            for a in range(RT):
                nc.tensor.matmul(b_ps[:, 128 * a:128 * (a + 1)],
                                 lhsT=ones_k1[:], rhs=yt_sb[:, a, :],
                                 start=True, stop=True)
            invn = inv_pool.tile([128, CHUNK], BF16)
            nc.vector.tensor_copy(invn[:], b_ps[:])
            xnc = xnc_pool.tile([128, KT, CHUNK], F8)
            for k in range(KT):
                nc.vector.tensor_mul(xnc[:, k, :], xbf[:, k, :], invn[:])
            return xnc

        def sweep(j, xnc):
            """All m-tiles against normalized chunk j; fused softmax stats."""
            for m in range(MT):
                g = ps_g.tile([128, CHUNK], F32)
                for t in range(KT // 2):
                    nc.tensor.matmul(
                        g[:], lhsT=xf8_own[:, 2 * t:2 * t + 2, 128 * m:128 * (m + 1)],
                        rhs=xnc[:, 2 * t:2 * t + 2, :],
                        start=(t == 0), stop=(t == KT // 2 - 1),
                        perf_mode=mybir.MatmulPerfMode.DoubleRow,
                    )
                esb = exp_pool.tile([128, CHUNK], BF16)
                nc.scalar.activation(
                    esb[:], g[:], EXP, scale=inv2_rm[:, m:m + 1],
                    accum_out=esum[:, m, j:j + 1],
                )
                if j == m // 4:
                    off = (m % 4) * 128
                    scr = scr_pool.tile([128, 128], BF16)
                    nc.vector.tensor_mul(
                        scr[:], esb[:, off:off + 128], mask_sb[:, 0:128])
                    nc.vector.tensor_reduce(
                        ediag[:, m:m + 1], scr[:],
                        axis=mybir.AxisListType.X, op=ADD)
                    scr2 = scr_pool.tile([128, 128], BF16)
                    nc.vector.tensor_mul(
                        scr2[:], esb[:, off:off + 128], mask_sb[:, 128:256])
                    nc.vector.tensor_reduce(
                        etarg[:, m:m + 1], scr2[:],
                        axis=mybir.AxisListType.X, op=ADD)
                if CS0 <= j < CS1:
                    nc.gpsimd.tensor_add(
                        cs_acc[:, j - CS0, :], cs_acc[:, j - CS0, :], esb[:])

        # Software pipeline: column/row staging runs PRE chunks ahead of
        # the sweep; normalize runs NLOOK ahead.  The PE stream is sweep
        # matmuls only, so nothing long-latency can head-of-line block it.
        # Prologue interleaves cm(j) | rm(j) | norm(j) so the chunk-0
        # chain (rm DMA -> sq -> newton -> scatter+load -> bcast -> muls)
        # reaches the first matmul with nothing queued in front of it.
        xbf_chunks = {}
        inv_chunks = {}
        xnc_chunks = {}
        for j in range(2):
            inv_chunks[j] = rm_stage(j)
            xbf_chunks[j] = stage_cm(j)
            xnc_chunks[j] = norm_chunk(j, xbf_chunks.pop(j), inv_chunks.pop(j))
        for j in range(2, PRE):
            inv_chunks[j] = rm_stage(j)
            xbf_chunks[j] = stage_cm(j)
        for jj in range(NCS):
            nc.gpsimd.memset(cs_acc[:, jj, :], 0.0)
        for j in range(NCH):
            sweep(j, xnc_chunks.pop(j))
            jms = [2, 3] if j == 0 else [j + NLOOK]
            for jm in jms:
                if jm < NCH:
                    xnc_chunks[jm] = norm_chunk(
                        jm, xbf_chunks.pop(jm), inv_chunks.pop(jm))
            jn = j + PRE
            if jn < NCH:
                inv_chunks[jn] = rm_stage(jn)
                xbf_chunks[jn] = stage_cm(jn)

        nc.sync.dma_start(esum_out[:], esum[:])
        nc.sync.dma_start(ediag_out[:], ediag[:])
        nc.sync.dma_start(etarg_out[:], etarg[:])
        nc.sync.dma_start(cs_out[:], cs_acc[:])

    nc.finalize()
    return nc


def _get_program():
    if "nc" not in _NC_CACHE:
        _NC_CACHE["nc"] = _build_program()
    return _NC_CACHE["nc"]


def _make_masks():
    m = np.zeros((128, 256), dtype=np.float32)
    p = np.arange(128)
    m[p, p] = 1.0              # identity: diagonal extraction
    m[p, 128 + (p ^ 1)] = 1.0  # pair-swap: target extraction
    return m.astype(BF)


def kernel(z_i: np.ndarray, z_j: np.ndarray, _trace: bool = False) -> np.ndarray:
    global LAST_RESULTS
    nc = _get_program()

    x = np.concatenate([np.asarray(z_i), np.asarray(z_j)], axis=0)
    assert x.shape == (N, D) and x.dtype == np.float32
    xb = x.astype(BF)                            # [8192, 1024] bf16
    xtb = np.ascontiguousarray(xb.T)             # [1024, 8192] bf16
    xg = xb.reshape(N // 128, 128, D)            # [64, 128, 1024]
    masks = _make_masks()

    in_maps = []
    xt8b = x.T.astype(F8NP)
    for c in range(NCORES):
        cols = (np.arange(NCOL) + RPC * c) % N
        xt_c = np.ascontiguousarray(xtb[:, cols])
        xt8_c = np.ascontiguousarray(xt8b[:, RPC * c:RPC * (c + 1)])
        rows_t = (np.arange(NRT) + (RPC // 128) * c) % (N // 128)
        xr_c = np.ascontiguousarray(xg[rows_t].transpose(1, 0, 2))
        in_maps.append({"xt": xt_c, "xt8": xt8_c, "xr": xr_c, "masks": masks})

    res = run_bass_kernel_spmd(
        nc, in_maps, core_ids=list(range(NCORES)), trace=_trace,
    )
    LAST_RESULTS = res

    # Host epilogue (O(N) numpy, f64): combine row partials with the
    # symmetric colsum partials, then mean(log denom - log E_target).
    denom = np.zeros(N, dtype=np.float64)
    ediag = np.zeros(N, dtype=np.float64)
    etarg = np.zeros(N, dtype=np.float64)
    pm = (128 * np.arange(MT)[None, :] + np.arange(128)[:, None]).ravel()  # row of [p, m]
    for c in range(NCORES):
        r = res.results[c]
        rows = RPC * c + pm
        denom[rows] += r["esum"].astype(np.float64).sum(axis=2).ravel()
        ediag[rows] = r["ediag"].astype(np.float64).ravel()
        etarg[rows] = r["etarg"].astype(np.float64).ravel()
        cs = r["colsums"].astype(np.float64).sum(axis=0).ravel()  # [NCS*CHUNK]
        gcols = (RPC * c + CS0 * CHUNK + np.arange(NCS * CHUNK)) % N
        denom[gcols] += cs
    loss = np.mean(np.log(denom - ediag) - np.log(etarg))
    return np.float32(loss)



# revision 2
# speedup vs baseline: 1.9684x; 1.9684x over previous
"""NT-Xent loss on 8 Trainium2 NeuronCores (Bass/Tile), v3.

Reference computation (B=4096, D=1024, T=0.5):
    x  = concat(z_i, z_j)                      # [8192, 1024] f32
    xn = x / ||x||                             # row-normalize
    sim = xn @ xn.T                            # [8192, 8192]
    logits = sim / T, diag masked to -inf
    loss = -mean(log_softmax(logits)[i, target(i)]), target(i) = i ^ 1

E = exp(sim/T) is symmetric, so only half the matrix need be computed.
Core c owns rows [1024c, 1024(c+1)) and sweeps 4.5 of the 8 column
blocks: blocks c..c+3 (mod 8) in full, plus half of block c+4 split by
quadrant between the two endpoint cores (cores 0-3 take the diagonal
quadrants Q00/Q11 of their d=4 block, cores 4-7 the anti-diagonal
Q01/Q10 of theirs) -- together an exact single cover of the matrix.
Row sums come from the ACT exp accumulator; the transposed halves are
covered by DVE column-sum accumulators shipped to the host (blocks
d=1..3 for all rows, plus the two m-restricted d=4 quadrant sums).
The host adds partials, subtracts the diagonal, and takes
mean(log(denom) - log(E_target)) in f64 -- an O(N) numpy epilogue.

v2 lesson (trace): the on-device normalization pipeline (row-major
re-load of x, DVE square+reduce, Newton rsqrt, PE transpose broadcast)
put the DVE at 95.7% busy and left the PE stalling at 32%/HAM-cold.
v3 normalizes on the host (O(N*D) staging prep, like the transpose +
fp8 casts that were already there) and ships fp8 on BOTH matmul sides:
the device does nothing but the DoubleRow fp8 sweep (PE), one
[128,2048]-wide exp per (group, m-tile) out of a 4-bank PSUM tile
(ACT), and bf16 colsum adds + diag/target extraction (DVE).
"""

import numpy as np
import ml_dtypes
from contextlib import ExitStack

import concourse.bass as bass
import concourse.tile as tile
from concourse import bacc, mybir
from concourse.bass_utils import run_bass_kernel_spmd

F32 = mybir.dt.float32
BF16 = mybir.dt.bfloat16
F8 = mybir.dt.float8e4
BF = ml_dtypes.bfloat16
F8NP = ml_dtypes.float8_e4m3
F8SCALE = 16.0
EXPSCALE = 2.0 / (F8SCALE * F8SCALE)   # exp((16 xn_r . 16 xn_c) / (256 T))

B = 4096
D = 1024
N = 2 * B            # 8192 rows total
NCORES = 8
RPC = N // NCORES    # 1024 rows per core
KT = D // 128        # 8 contraction partition-tiles
MT = RPC // 128      # 8 row tiles per core
CHUNK = 512
NREG = 4 * RPC       # regular swept columns (blocks d=0..3)
NG2 = RPC            # staged d=4 columns (chunk 8 for m<4, chunk 9 for m>=4)
NCOL = NREG + NG2    # 5120 staged columns
NCH = NCOL // CHUNK  # 10 column chunks
NCS = 6              # chunks whose colsums ship (d=1..3)

_NC_CACHE = {}
LAST_RESULTS = None  # BassKernelResults of the most recent run (for test.py)


def _build_program():
    nc = bacc.Bacc("TRN2", target_bir_lowering=False, debug=False)

    xall8 = nc.dram_tensor("xall8", [NCH, 128, KT, CHUNK], F8, kind="ExternalInput")
    xown8 = nc.dram_tensor("xown8", [128, KT, RPC], F8, kind="ExternalInput")
    masks = nc.dram_tensor("masks", [128, 256], BF16, kind="ExternalInput")
    esum_out = nc.dram_tensor("esum", [128, MT, 3], F32, kind="ExternalOutput")
    ediag_out = nc.dram_tensor("ediag", [128, MT], F32, kind="ExternalOutput")
    etarg_out = nc.dram_tensor("etarg", [128, MT], F32, kind="ExternalOutput")
    cs_out = nc.dram_tensor("colsums", [128, NCS * CHUNK + NG2], BF16,
                            kind="ExternalOutput")

    ADD = mybir.AluOpType.add
    EXP = mybir.ActivationFunctionType.Exp

    with tile.TileContext(nc) as tc, ExitStack() as ctx:
        consts = ctx.enter_context(tc.tile_pool(name="consts", bufs=1))
        xin_pool = ctx.enter_context(tc.tile_pool(name="xin", bufs=1))
        exp_pool = ctx.enter_context(tc.tile_pool(name="exp", bufs=3))
        scr_pool = ctx.enter_context(tc.tile_pool(name="scr", bufs=2))
        stat_pool = ctx.enter_context(tc.tile_pool(name="stat", bufs=1))
        ps_pool = ctx.enter_context(tc.tile_pool(name="ps", bufs=2, space="PSUM"))
        # 2 x 4-bank PSUM tiles == all 8 banks

        mask_sb = consts.tile([128, 256], BF16)
        nc.scalar.dma_start(mask_sb[:], masks[:])

        xown = xin_pool.tile([128, KT, RPC], F8)
        nc.scalar.dma_start(xown[:, :, 0:RPC // 2], xown8[:, :, 0:RPC // 2])
        nc.scalar.dma_start(xown[:, :, RPC // 2:RPC], xown8[:, :, RPC // 2:RPC])

        xall = xin_pool.tile([128, NCH, KT, CHUNK], F8)
        for j in range(NCH):
            eng = nc.sync if j % 2 == 0 else nc.gpsimd
            eng.dma_start(xall[:, j], xall8[j])

        esum = stat_pool.tile([128, MT, 3], F32)
        ediag = stat_pool.tile([128, MT], F32)
        etarg = stat_pool.tile([128, MT], F32)
        cs13 = stat_pool.tile([128, NCS * CHUNK], BF16)
        cs4a = stat_pool.tile([128, CHUNK], BF16)
        cs4b = stat_pool.tile([128, CHUNK], BF16)
        nc.vector.memset(cs13[:], 0.0)
        nc.vector.memset(cs4a[:], 0.0)
        nc.vector.memset(cs4b[:], 0.0)

        # Sweep: group-outer (G0 = chunks 0-3, G1 = 4-7, G2 = half-chunk
        # per m-half) so the rhs DMA for a group lands a full group of PE
        # work (~31 us) before it's needed.  m inner; chunk innermost so
        # each DoubleRow weight load feeds 4 consecutive matmuls.
        for g in range(3):
            for m in range(MT):
                chunks = range(4 * g, 4 * g + 4) if g < 2 else \
                    [8 if m < 4 else 9]
                w = CHUNK * len(chunks)
                ps = ps_pool.tile([128, 2048], F32)
                for t in range(KT // 2):
                    for ci, j in enumerate(chunks):
                        nc.tensor.matmul(
                            ps[:, CHUNK * ci:CHUNK * (ci + 1)],
                            lhsT=xown[:, 2 * t:2 * t + 2, 128 * m:128 * (m + 1)],
                            rhs=xall[:, j, 2 * t:2 * t + 2, :],
                            start=(t == 0), stop=(t == KT // 2 - 1),
                            perf_mode=mybir.MatmulPerfMode.DoubleRow,
                        )
                esb = exp_pool.tile([128, w], BF16)
                nc.scalar.activation(
                    esb[:], ps[:, 0:w], EXP, scale=EXPSCALE,
                    accum_out=esum[:, m, g:g + 1],
                )
                if g == 0:
                    off = 128 * m
                    scr = scr_pool.tile([128, 128], BF16)
                    nc.vector.tensor_mul(
                        scr[:], esb[:, off:off + 128], mask_sb[:, 0:128])
                    nc.vector.tensor_reduce(
                        ediag[:, m:m + 1], scr[:],
                        axis=mybir.AxisListType.X, op=ADD)
                    scr2 = scr_pool.tile([128, 128], BF16)
                    nc.vector.tensor_mul(
                        scr2[:], esb[:, off:off + 128], mask_sb[:, 128:256])
                    nc.vector.tensor_reduce(
                        etarg[:, m:m + 1], scr2[:],
                        axis=mybir.AxisListType.X, op=ADD)
                    nc.vector.tensor_add(
                        cs13[:, 0:1024], cs13[:, 0:1024], esb[:, 1024:2048])
                elif g == 1:
                    nc.vector.tensor_add(cs13[:, 1024:3072],
                                         cs13[:, 1024:3072], esb[:])
                else:
                    tgt = cs4a if m < 4 else cs4b
                    nc.vector.tensor_add(tgt[:], tgt[:], esb[:])

        nc.sync.dma_start(esum_out[:], esum[:])
        nc.sync.dma_start(ediag_out[:], ediag[:])
        nc.sync.dma_start(etarg_out[:], etarg[:])
        nc.sync.dma_start(cs_out[:, 0:NCS * CHUNK], cs13[:])
        nc.sync.dma_start(cs_out[:, NCS * CHUNK:NCS * CHUNK + CHUNK], cs4a[:])
        nc.sync.dma_start(cs_out[:, NCS * CHUNK + CHUNK:], cs4b[:])

    nc.finalize()
    return nc


def _get_program():
    if "nc" not in _NC_CACHE:
        _NC_CACHE["nc"] = _build_program()
    return _NC_CACHE["nc"]


def _make_masks():
    m = np.zeros((128, 256), dtype=np.float32)
    p = np.arange(128)
    m[p, p] = 1.0              # identity: diagonal extraction
    m[p, 128 + (p ^ 1)] = 1.0  # pair-swap: target extraction
    return m.astype(BF)


def kernel(z_i: np.ndarray, z_j: np.ndarray, _trace: bool = False) -> np.ndarray:
    global LAST_RESULTS
    nc = _get_program()

    x = np.concatenate([np.asarray(z_i), np.asarray(z_j)], axis=0)
    assert x.shape == (N, D) and x.dtype == np.float32
    xn = x / np.maximum(np.sqrt((x.astype(np.float64) ** 2).sum(axis=1,
                        keepdims=True)), 1e-8)
    x8 = (xn * F8SCALE).astype(F8NP)             # [8192, 1024] fp8
    x8t = np.ascontiguousarray(x8.T)             # [1024, 8192] fp8
    masks = _make_masks()

    in_maps = []
    for c in range(NCORES):
        b = ((c + 4) % NCORES) * RPC
        if c < 4:
            g2cols = b + np.arange(NG2)
        else:
            g2cols = b + (np.arange(NG2) + 512) % NG2
        cols = np.concatenate([(c * RPC + np.arange(NREG)) % N, g2cols])
        # chunk-major staging: [NCH, 128, KT, CHUNK], contiguous per
        # (chunk, partition) so each chunk DMA is one 4KB run/partition
        xt_c = x8t[:, cols].reshape(KT, 128, NCH, CHUNK)
        xall_c = np.ascontiguousarray(xt_c.transpose(2, 1, 0, 3))
        xown_c = np.ascontiguousarray(
            x8t[:, c * RPC:(c + 1) * RPC].reshape(KT, 128, RPC)
            .transpose(1, 0, 2))
        in_maps.append({"xall8": xall_c, "xown8": xown_c, "masks": masks})

    res = run_bass_kernel_spmd(
        nc, in_maps, core_ids=list(range(NCORES)), trace=_trace,
    )
    LAST_RESULTS = res

    # Host epilogue (O(N) numpy, f64): combine row partials with the
    # symmetric colsum partials, then mean(log denom - log E_target).
    denom = np.zeros(N, dtype=np.float64)
    ediag = np.zeros(N, dtype=np.float64)
    etarg = np.zeros(N, dtype=np.float64)
    pm = (128 * np.arange(MT)[None, :] + np.arange(128)[:, None]).ravel()
    for c in range(NCORES):
        r = res.results[c]
        rows = RPC * c + pm
        denom[rows] += r["esum"].astype(np.float64).sum(axis=2).ravel()
        ediag[rows] = r["ediag"].astype(np.float64).ravel()
        etarg[rows] = r["etarg"].astype(np.float64).ravel()
        cs = r["colsums"].astype(np.float64).sum(axis=0)  # [NCS*CHUNK + NG2]
        gcols = (RPC * c + RPC + np.arange(NCS * CHUNK)) % N
        denom[gcols] += cs[0:NCS * CHUNK]
        b = ((c + 4) % NCORES) * RPC
        if c < 4:
            g2cols = b + np.arange(NG2)
        else:
            g2cols = b + (np.arange(NG2) + 512) % NG2
        denom[g2cols[0:CHUNK]] += cs[NCS * CHUNK:NCS * CHUNK + CHUNK]
        denom[g2cols[CHUNK:]] += cs[NCS * CHUNK + CHUNK:]
    loss = np.mean(np.log(denom - ediag) - np.log(etarg))
    return np.float32(loss)


# revision 6
# speedup vs baseline: 2.2278x; 1.1318x over previous
"""NT-Xent loss on 8 Trainium2 NeuronCores (Bass/Tile), v4.

Reference computation (B=4096, D=1024, T=0.5):
    x  = concat(z_i, z_j)                      # [8192, 1024] f32
    xn = x / ||x||                             # row-normalize
    sim = xn @ xn.T                            # [8192, 8192]
    logits = sim / T, diag masked to -inf
    loss = -mean(log_softmax(logits)[i, target(i)]), target(i) = i ^ 1

E = exp(sim/T) is symmetric, so only half the matrix need be computed.
Core c owns rows [1024c, 1024(c+1)) and sweeps 4.5 of the 8 column
blocks: blocks c..c+3 (mod 8) in full, plus half of block c+4 split by
quadrant between the two endpoint cores (cores 0-3 take the diagonal
quadrants Q00/Q11 of their d=4 block, cores 4-7 the anti-diagonal
Q01/Q10) -- together an exact single cover.  Row sums come from the
ACT exp accumulator; transposed halves are covered by DVE column-sum
accumulators shipped to the host (blocks d=1..3 for all rows, plus two
m-half-restricted d=4 quadrant sums).  The host combines partials,
subtracts the diagonal, and takes mean(log denom - log E_target) in
f64 -- an O(N) numpy epilogue.

Normalization happens on the host (O(N*D) staging prep, like the
transpose + fp8 casts); both matmul sides are fp8 at scale 16, so the
device is a pure DoubleRow-fp8 sweep (PE) + one wide exp per
(phase, m-tile) out of a multi-bank PSUM tile (ACT) + bf16 colsum adds
and diag/target extraction (DVE).

v3 lesson (trace): the sweep itself ran at the fp8 roofline (216 ns/MM
warm, one 1.8 us stall) but the first matmul waited 29 us for ALL
input DMAs -- the single resident rhs tile made every MM depend on
every chunk DMA.  v4 stages per-chunk/per-m-tile tiles (fine-grained
deps), orders the DMA queues by first-use (chunk 0 + m-tile-0 weights
land in ~2 us), sweeps chunk-count-increasing phases [c0][c1][c2-3]
[c4-7][c8/9] so each phase's rhs arrives during the previous phases,
and warms the PE HAM clock-gate with dummy matmuls during the DMA
prologue.
"""

import numpy as np
import ml_dtypes
from contextlib import ExitStack

import concourse.bass as bass
import concourse.tile as tile
from concourse import bacc, mybir
from concourse.bass_utils import run_bass_kernel_spmd

F32 = mybir.dt.float32
BF16 = mybir.dt.bfloat16
F8 = mybir.dt.float8e4
BF = ml_dtypes.bfloat16
F8NP = ml_dtypes.float8_e4m3
F8SCALE = 16.0
EXPSCALE = 2.0 / (F8SCALE * F8SCALE)   # exp((16 xn_r . 16 xn_c) / (256 T))

B = 4096
D = 1024
N = 2 * B            # 8192 rows total
NCORES = 8
RPC = N // NCORES    # 1024 rows per core
KT = D // 128        # 8 contraction partition-tiles
MT = RPC // 128      # 8 row tiles per core
CHUNK = 512
NREG = 4 * RPC       # regular swept columns (blocks d=0..3)
NG2 = RPC            # staged d=4 columns (chunk 8 for m<4, chunk 9 for m>=4)
NCOL = NREG + NG2    # 5120 staged columns
NCH = NCOL // CHUNK  # 10 column chunks
NCS = 6              # chunks whose colsums ship (d=1..3)
NWARM = 10           # dummy PE warm-up matmuls during the DMA prologue

# chunk phases: each phase sweeps all 8 m-tiles over these chunks
PHASES = [[0], [1], [2, 3], [4, 5, 6, 7], [-1]]   # -1: chunk 8 or 9 by m-half
NPH = len(PHASES)

_NC_CACHE = {}
LAST_RESULTS = None  # BassKernelResults of the most recent run (for test.py)


def _build_program():
    nc = bacc.Bacc("TRN2", target_bir_lowering=False, debug=False)

    xall8 = nc.dram_tensor("xall8", [NCH, 128, KT, CHUNK], F8, kind="ExternalInput")
    xown8 = nc.dram_tensor("xown8", [MT, 128, KT, 128], F8, kind="ExternalInput")
    masks = nc.dram_tensor("masks", [128, 256], BF16, kind="ExternalInput")
    esum_out = nc.dram_tensor("esum", [128, MT, NPH], F32, kind="ExternalOutput")
    ediag_out = nc.dram_tensor("ediag", [128, MT], F32, kind="ExternalOutput")
    etarg_out = nc.dram_tensor("etarg", [128, MT], F32, kind="ExternalOutput")
    cs_out = nc.dram_tensor("colsums", [128, NCS * CHUNK + NG2], BF16,
                            kind="ExternalOutput")

    ADD = mybir.AluOpType.add
    EXP = mybir.ActivationFunctionType.Exp

    with tile.TileContext(nc) as tc, ExitStack() as ctx:
        consts = ctx.enter_context(tc.tile_pool(name="consts", bufs=1))
        xin_pool = ctx.enter_context(tc.tile_pool(name="xin", bufs=1))
        exp_pool = ctx.enter_context(tc.tile_pool(name="exp", bufs=3))
        scr_pool = ctx.enter_context(tc.tile_pool(name="scr", bufs=2))
        stat_pool = ctx.enter_context(tc.tile_pool(name="stat", bufs=1))
        ps_pool = ctx.enter_context(tc.tile_pool(name="ps", bufs=2, space="PSUM"))
        # 2 x 4-bank PSUM tiles == all 8 banks

        # PE warm-up operand: memset'd zeros, no DMA dependency
        zbf = consts.tile([128, 640], BF16)
        nc.vector.memset(zbf[:], 0.0)

        mask_sb = consts.tile([128, 256], BF16)
        xc = [xin_pool.tile([128, KT, CHUNK], F8, name=f"xc{j}")
              for j in range(NCH)]
        xo = [xin_pool.tile([128, KT, 128], F8, name=f"xo{m}")
              for m in range(MT)]

        # DMA issue order by first use; chunk 0 split across two queues.
        nc.gpsimd.dma_start(mask_sb[:], masks[:])
        h = KT // 2
        nc.scalar.dma_start(xc[0][:, 0:h], xall8[0, :, 0:h])
        nc.sync.dma_start(xc[0][:, h:KT], xall8[0, :, h:KT])
        nc.gpsimd.dma_start(xo[0][:], xown8[0])
        nc.scalar.dma_start(xo[1][:], xown8[1])
        nc.sync.dma_start(xo[2][:], xown8[2])
        nc.gpsimd.dma_start(xo[3][:], xown8[3])
        nc.scalar.dma_start(xo[4][:], xown8[4])
        nc.sync.dma_start(xo[5][:], xown8[5])
        nc.gpsimd.dma_start(xo[6][:], xown8[6])
        nc.scalar.dma_start(xo[7][:], xown8[7])
        nc.sync.dma_start(xc[1][:], xall8[1])
        nc.gpsimd.dma_start(xc[4][:], xall8[4])
        nc.scalar.dma_start(xc[2][:], xall8[2])
        nc.sync.dma_start(xc[3][:], xall8[3])
        nc.gpsimd.dma_start(xc[7][:], xall8[7])
        nc.scalar.dma_start(xc[5][:], xall8[5])
        nc.sync.dma_start(xc[6][:], xall8[6])
        nc.scalar.dma_start(xc[8][:], xall8[8])
        nc.sync.dma_start(xc[9][:], xall8[9])

        esum = stat_pool.tile([128, MT, NPH], F32)
        ediag = stat_pool.tile([128, MT], F32)
        etarg = stat_pool.tile([128, MT], F32)
        cs13 = stat_pool.tile([128, NCS * CHUNK], BF16)
        cs4a = stat_pool.tile([128, CHUNK], BF16)
        cs4b = stat_pool.tile([128, CHUNK], BF16)
        nc.vector.memset(cs13[:], 0.0)
        nc.vector.memset(cs4a[:], 0.0)
        nc.vector.memset(cs4b[:], 0.0)

        # HAM warm-up: dummy bf16 matmuls fill the otherwise-idle PE
        # during the first chunk's DMA so the 2.4 GHz clock-gate opens
        # before the real sweep starts.
        ps_w = ps_pool.tile([128, 2048], F32, name="ps", tag="ps")
        for _ in range(NWARM):
            nc.tensor.matmul(ps_w[:, 0:CHUNK], lhsT=zbf[:, 0:128],
                             rhs=zbf[:, 128:640], start=True, stop=True)

        for ph, chunks in enumerate(PHASES):
            for m in range(MT):
                cj = [(8 if m < 4 else 9) if j < 0 else j for j in chunks]
                w = CHUNK * len(cj)
                ps = ps_pool.tile([128, 2048], F32, name="ps", tag="ps")
                for t in range(KT // 2):
                    for ci, j in enumerate(cj):
                        nc.tensor.matmul(
                            ps[:, CHUNK * ci:CHUNK * (ci + 1)],
                            lhsT=xo[m][:, 2 * t:2 * t + 2, :],
                            rhs=xc[j][:, 2 * t:2 * t + 2, :],
                            start=(t == 0), stop=(t == KT // 2 - 1),
                            perf_mode=mybir.MatmulPerfMode.DoubleRow,
                        )
                esb = exp_pool.tile([128, w], BF16)
                nc.scalar.activation(
                    esb[:], ps[:, 0:w], EXP, scale=EXPSCALE,
                    accum_out=esum[:, m, ph:ph + 1],
                )
                if (ph == 0 and m < 4) or (ph == 1 and m >= 4):
                    off = 128 * m - (0 if ph == 0 else CHUNK)
                    scr = scr_pool.tile([128, 128], BF16)
                    nc.vector.tensor_mul(
                        scr[:], esb[:, off:off + 128], mask_sb[:, 0:128])
                    nc.vector.tensor_reduce(
                        ediag[:, m:m + 1], scr[:],
                        axis=mybir.AxisListType.X, op=ADD)
                    scr2 = scr_pool.tile([128, 128], BF16)
                    nc.vector.tensor_mul(
                        scr2[:], esb[:, off:off + 128], mask_sb[:, 128:256])
                    nc.vector.tensor_reduce(
                        etarg[:, m:m + 1], scr2[:],
                        axis=mybir.AxisListType.X, op=ADD)
                elif ph == 2:
                    nc.vector.tensor_add(cs13[:, 0:1024], cs13[:, 0:1024],
                                         esb[:])
                elif ph == 3:
                    nc.vector.tensor_add(cs13[:, 1024:3072],
                                         cs13[:, 1024:3072], esb[:])
                elif ph == 4:
                    tgt = cs4a if m < 4 else cs4b
                    nc.vector.tensor_add(tgt[:], tgt[:], esb[:])

        nc.sync.dma_start(esum_out[:], esum[:])
        nc.sync.dma_start(ediag_out[:], ediag[:])
        nc.sync.dma_start(etarg_out[:], etarg[:])
        nc.sync.dma_start(cs_out[:, 0:NCS * CHUNK], cs13[:])
        nc.sync.dma_start(cs_out[:, NCS * CHUNK:NCS * CHUNK + CHUNK], cs4a[:])
        nc.sync.dma_start(cs_out[:, NCS * CHUNK + CHUNK:], cs4b[:])

    nc.finalize()
    return nc


def _get_program():
    if "nc" not in _NC_CACHE:
        _NC_CACHE["nc"] = _build_program()
    return _NC_CACHE["nc"]


def _make_masks():
    m = np.zeros((128, 256), dtype=np.float32)
    p = np.arange(128)
    m[p, p] = 1.0              # identity: diagonal extraction
    m[p, 128 + (p ^ 1)] = 1.0  # pair-swap: target extraction
    return m.astype(BF)


def kernel(z_i: np.ndarray, z_j: np.ndarray, _trace: bool = False) -> np.ndarray:
    global LAST_RESULTS
    nc = _get_program()

    x = np.concatenate([np.asarray(z_i), np.asarray(z_j)], axis=0)
    assert x.shape == (N, D) and x.dtype == np.float32
    xn = x / np.maximum(np.sqrt((x.astype(np.float64) ** 2).sum(axis=1,
                        keepdims=True)), 1e-8)
    x8 = (xn * F8SCALE).astype(F8NP)             # [8192, 1024] fp8
    x8t = np.ascontiguousarray(x8.T)             # [1024, 8192] fp8
    masks = _make_masks()

    in_maps = []
    for c in range(NCORES):
        b = ((c + 4) % NCORES) * RPC
        if c < 4:
            g2cols = b + np.arange(NG2)
        else:
            g2cols = b + (np.arange(NG2) + 512) % NG2
        cols = np.concatenate([(c * RPC + np.arange(NREG)) % N, g2cols])
        # chunk-major staging: [NCH, 128, KT, CHUNK], contiguous per
        # (chunk, partition) so each chunk DMA is one 4KB run/partition
        xt_c = x8t[:, cols].reshape(KT, 128, NCH, CHUNK)
        xall_c = np.ascontiguousarray(xt_c.transpose(2, 1, 0, 3))
        xown_c = np.ascontiguousarray(
            x8t[:, c * RPC:(c + 1) * RPC].reshape(KT, 128, MT, 128)
            .transpose(2, 1, 0, 3))
        in_maps.append({"xall8": xall_c, "xown8": xown_c, "masks": masks})

    res = run_bass_kernel_spmd(
        nc, in_maps, core_ids=list(range(NCORES)), trace=_trace,
    )
    LAST_RESULTS = res

    # Host epilogue (O(N) numpy, f64): combine row partials with the
    # symmetric colsum partials, then mean(log denom - log E_target).
    denom = np.zeros(N, dtype=np.float64)
    ediag = np.zeros(N, dtype=np.float64)
    etarg = np.zeros(N, dtype=np.float64)
    pm = (128 * np.arange(MT)[None, :] + np.arange(128)[:, None]).ravel()
    for c in range(NCORES):
        r = res.results[c]
        rows = RPC * c + pm
        denom[rows] += r["esum"].astype(np.float64).sum(axis=2).ravel()
        ediag[rows] = r["ediag"].astype(np.float64).ravel()
        etarg[rows] = r["etarg"].astype(np.float64).ravel()
        cs = r["colsums"].astype(np.float64).sum(axis=0)  # [NCS*CHUNK + NG2]
        gcols = (RPC * c + RPC + np.arange(NCS * CHUNK)) % N
        denom[gcols] += cs[0:NCS * CHUNK]
        b = ((c + 4) % NCORES) * RPC
        if c < 4:
            g2cols = b + np.arange(NG2)
        else:
            g2cols = b + (np.arange(NG2) + 512) % NG2
        denom[g2cols[0:CHUNK]] += cs[NCS * CHUNK:NCS * CHUNK + CHUNK]
        denom[g2cols[CHUNK:]] += cs[NCS * CHUNK + CHUNK:]
    loss = np.mean(np.log(denom - ediag) - np.log(etarg))
    return np.float32(loss)


# revision 8
# speedup vs baseline: 2.3994x; 1.0770x over previous
"""NT-Xent loss on 8 Trainium2 NeuronCores (Bass/Tile), v5.

Reference computation (B=4096, D=1024, T=0.5):
    x  = concat(z_i, z_j)                      # [8192, 1024] f32
    xn = x / ||x||                             # row-normalize
    sim = xn @ xn.T                            # [8192, 8192]
    logits = sim / T, diag masked to -inf
    loss = -mean(log_softmax(logits)[i, target(i)]), target(i) = i ^ 1

E = exp(sim/T) is symmetric, so only half the matrix need be computed.
Core c owns rows [1024c, 1024(c+1)) and sweeps ~4.25 of the 8 column
blocks:
  - blocks c+1..c+3 (mod 8) in full,
  - its own diagonal block minus the lower-left quadrant (the triangle
    trick: quadrant D10 is recovered from a column-sum of D01
    restricted to the top m-half),
  - half of block c+4, quadrant-split between the two endpoint cores
    (cores 0-3 take the diagonal quadrants Q00/Q11 of their d=4 block,
    cores 4-7 the anti-diagonal Q01/Q10)
-- together an exact single cover.  Row sums come from the ACT exp
accumulator; transposed halves are covered by DVE column-sum
accumulators shipped to the host.  The host combines partials,
subtracts the diagonal, and takes mean(log denom - log E_target) in
f64 -- an O(N) numpy epilogue.

Normalization happens on the host (O(N*D) staging prep, like the
transpose + fp8 casts); both matmul sides are fp8 at scale 16, sliced
from the same chunk tiles (the d=0 chunks ARE the own rows), so the
device is a pure DoubleRow-fp8 sweep (PE) + one wide exp per
(phase, m-tile) out of a multi-bank PSUM tile (ACT) + bf16 colsum adds
and diag/target extraction (DVE).

v4 lessons (trace): per-chunk tiles + queue-ordered DMA moved the
first matmul from 29 us to 8 us, and the HAM warm-up matmuls held the
PE at 2.4 GHz -- but the separate weight-tile DMAs ahead of chunk 1
starved phase B (3.1 us stall + a HAM cold dip), and the small-matmul
phase E stalled 1.8 us behind the 2 us activations of big phase D.
v5 slices weights from the chunk tiles (1 MB less DMA, chunk 1 lands
in time), runs the phases in chunk-count-increasing order
A(c0) B(c1) E(c8/9) C(c2-3) D(c4-7) so activation width only grows,
and drops the 16 triangle-trick matmuls.
"""

import numpy as np
import ml_dtypes
from contextlib import ExitStack

import concourse.bass as bass
import concourse.tile as tile
from concourse import bacc, mybir
from concourse.bass_utils import run_bass_kernel_spmd

F32 = mybir.dt.float32
BF16 = mybir.dt.bfloat16
F8 = mybir.dt.float8e4
BF = ml_dtypes.bfloat16
F8NP = ml_dtypes.float8_e4m3
F8SCALE = 16.0
EXPSCALE = 2.0 / (F8SCALE * F8SCALE)   # exp((16 xn_r . 16 xn_c) / (256 T))

B = 4096
D = 1024
N = 2 * B            # 8192 rows total
NCORES = 8
RPC = N // NCORES    # 1024 rows per core
KT = D // 128        # 8 contraction partition-tiles
MT = RPC // 128      # 8 row tiles per core
CHUNK = 512
NREG = 4 * RPC       # regular swept columns (blocks d=0..3)
NG2 = RPC            # staged d=4 columns (chunk 8 for m<4, chunk 9 for m>=4)
NCOL = NREG + NG2    # 5120 staged columns
NCH = NCOL // CHUNK  # 10 column chunks
NCS = 6              # chunks whose colsums ship (d=1..3)
NWARM = 10           # dummy PE warm-up matmuls during the DMA prologue

# phases: (chunk list, m range); -1 means chunk 8 or 9 by m-half.
# Phase A covers only the top m-half of chunk 0 (triangle trick).
PHASES = [
    ([0], range(0, 4)),
    ([1], range(MT)),
    ([-1], range(MT)),
    ([2, 3], range(MT)),
    ([4, 5, 6, 7], range(MT)),
]
NPH = len(PHASES)

_NC_CACHE = {}
LAST_RESULTS = None  # BassKernelResults of the most recent run (for test.py)


def _build_program():
    nc = bacc.Bacc("TRN2", target_bir_lowering=False, debug=False)

    xall8 = nc.dram_tensor("xall8", [NCH, 128, KT, CHUNK], F8, kind="ExternalInput")
    masks = nc.dram_tensor("masks", [128, 256], BF16, kind="ExternalInput")
    esum_out = nc.dram_tensor("esum", [128, MT, NPH], F32, kind="ExternalOutput")
    ediag_out = nc.dram_tensor("ediag", [128, MT], F32, kind="ExternalOutput")
    etarg_out = nc.dram_tensor("etarg", [128, MT], F32, kind="ExternalOutput")
    # [cs13 d=1..3 | cs4a | cs4b | cs_d01]
    cs_out = nc.dram_tensor("colsums", [128, NCS * CHUNK + NG2 + CHUNK], BF16,
                            kind="ExternalOutput")

    ADD = mybir.AluOpType.add
    EXP = mybir.ActivationFunctionType.Exp

    with tile.TileContext(nc) as tc, ExitStack() as ctx:
        consts = ctx.enter_context(tc.tile_pool(name="consts", bufs=1))
        xin_pool = ctx.enter_context(tc.tile_pool(name="xin", bufs=1))
        exp_pool = ctx.enter_context(tc.tile_pool(name="exp", bufs=3))
        scr_pool = ctx.enter_context(tc.tile_pool(name="scr", bufs=2))
        stat_pool = ctx.enter_context(tc.tile_pool(name="stat", bufs=1))
        ps_pool = ctx.enter_context(tc.tile_pool(name="ps", bufs=2, space="PSUM"))
        # 2 x 4-bank PSUM tiles == all 8 banks

        # PE warm-up operand: memset'd zeros, no DMA dependency
        zbf = consts.tile([128, 640], BF16)
        nc.vector.memset(zbf[:], 0.0)

        mask_sb = consts.tile([128, 256], BF16)
        xc = [xin_pool.tile([128, KT, CHUNK], F8, name=f"xc{j}")
              for j in range(NCH)]

        # DMA issue order by first use: chunk 0 quartered across four
        # queues, chunk 1 halved, then chunks 8/9 (phase E is third),
        # then the rest round-robin.
        nc.scalar.dma_start(xc[0][:, 0:3], xall8[0, :, 0:3])
        nc.sync.dma_start(xc[0][:, 3:6], xall8[0, :, 3:6])
        nc.gpsimd.dma_start(xc[0][:, 6:KT], xall8[0, :, 6:KT])
        h = KT // 2
        nc.scalar.dma_start(xc[1][:, 0:h], xall8[1, :, 0:h])
        nc.sync.dma_start(xc[1][:, h:KT], xall8[1, :, h:KT])
        nc.gpsimd.dma_start(mask_sb[:], masks[:])
        nc.gpsimd.dma_start(xc[8][:], xall8[8])
        nc.scalar.dma_start(xc[9][:], xall8[9])
        nc.sync.dma_start(xc[2][:], xall8[2])
        nc.gpsimd.dma_start(xc[3][:], xall8[3])
        nc.scalar.dma_start(xc[4][:], xall8[4])
        nc.sync.dma_start(xc[5][:], xall8[5])
        nc.gpsimd.dma_start(xc[6][:], xall8[6])
        nc.scalar.dma_start(xc[7][:], xall8[7])

        esum = stat_pool.tile([128, MT, NPH], F32)
        ediag = stat_pool.tile([128, MT], F32)
        etarg = stat_pool.tile([128, MT], F32)
        cs13 = stat_pool.tile([128, NCS * CHUNK], BF16)
        cs4a = stat_pool.tile([128, CHUNK], BF16)
        cs4b = stat_pool.tile([128, CHUNK], BF16)
        csd01 = stat_pool.tile([128, CHUNK], BF16)
        nc.vector.memset(esum[:], 0.0)
        nc.vector.memset(cs13[:], 0.0)
        nc.vector.memset(cs4a[:], 0.0)
        nc.vector.memset(cs4b[:], 0.0)
        nc.vector.memset(csd01[:], 0.0)

        # HAM warm-up: dummy bf16 matmuls fill the otherwise-idle PE
        # during the first chunk's DMA so the 2.4 GHz clock-gate opens
        # before the real sweep starts.
        ps_w = ps_pool.tile([128, 2048], F32, name="ps", tag="ps")
        for _ in range(NWARM):
            nc.tensor.matmul(ps_w[:, 0:CHUNK], lhsT=zbf[:, 0:128],
                             rhs=zbf[:, 128:640], start=True, stop=True)

        for ph, (chunks, ms) in enumerate(PHASES):
            for m in ms:
                cj = [(8 if m < 4 else 9) if j < 0 else j for j in chunks]
                w = CHUNK * len(cj)
                ps = ps_pool.tile([128, 2048], F32, name="ps", tag="ps")
                for t in range(KT // 2):
                    for ci, j in enumerate(cj):
                        nc.tensor.matmul(
                            ps[:, CHUNK * ci:CHUNK * (ci + 1)],
                            lhsT=xc[m // 4][:, 2 * t:2 * t + 2,
                                            128 * (m % 4):128 * (m % 4) + 128],
                            rhs=xc[j][:, 2 * t:2 * t + 2, :],
                            start=(t == 0), stop=(t == KT // 2 - 1),
                            perf_mode=mybir.MatmulPerfMode.DoubleRow,
                        )
                esb = exp_pool.tile([128, w], BF16)
                nc.scalar.activation(
                    esb[:], ps[:, 0:w], EXP, scale=EXPSCALE,
                    accum_out=esum[:, m, ph:ph + 1],
                )
                if (ph == 0 and m < 4) or (ph == 1 and m >= 4):
                    off = 128 * m - (0 if ph == 0 else CHUNK)
                    scr = scr_pool.tile([128, 128], BF16)
                    nc.vector.tensor_mul(
                        scr[:], esb[:, off:off + 128], mask_sb[:, 0:128])
                    nc.vector.tensor_reduce(
                        ediag[:, m:m + 1], scr[:],
                        axis=mybir.AxisListType.X, op=ADD)
                    scr2 = scr_pool.tile([128, 128], BF16)
                    nc.vector.tensor_mul(
                        scr2[:], esb[:, off:off + 128], mask_sb[:, 128:256])
                    nc.vector.tensor_reduce(
                        etarg[:, m:m + 1], scr2[:],
                        axis=mybir.AxisListType.X, op=ADD)
                if ph == 1 and m < 4:
                    nc.vector.tensor_add(csd01[:], csd01[:], esb[:])
                elif ph == 2:
                    tgt = cs4a if m < 4 else cs4b
                    nc.vector.tensor_add(tgt[:], tgt[:], esb[:])
                elif ph == 3:
                    nc.vector.tensor_add(cs13[:, 0:1024], cs13[:, 0:1024],
                                         esb[:])
                elif ph == 4:
                    nc.vector.tensor_add(cs13[:, 1024:3072],
                                         cs13[:, 1024:3072], esb[:])

        nc.sync.dma_start(esum_out[:], esum[:])
        nc.sync.dma_start(ediag_out[:], ediag[:])
        nc.sync.dma_start(etarg_out[:], etarg[:])
        nc.sync.dma_start(cs_out[:, 0:NCS * CHUNK], cs13[:])
        s = NCS * CHUNK
        nc.sync.dma_start(cs_out[:, s:s + CHUNK], cs4a[:])
        nc.sync.dma_start(cs_out[:, s + CHUNK:s + 2 * CHUNK], cs4b[:])
        nc.sync.dma_start(cs_out[:, s + 2 * CHUNK:], csd01[:])

    nc.finalize()
    return nc


def _get_program():
    if "nc" not in _NC_CACHE:
        _NC_CACHE["nc"] = _build_program()
    return _NC_CACHE["nc"]


def _make_masks():
    m = np.zeros((128, 256), dtype=np.float32)
    p = np.arange(128)
    m[p, p] = 1.0              # identity: diagonal extraction
    m[p, 128 + (p ^ 1)] = 1.0  # pair-swap: target extraction
    return m.astype(BF)


def kernel(z_i: np.ndarray, z_j: np.ndarray, _trace: bool = False) -> np.ndarray:
    global LAST_RESULTS
    nc = _get_program()

    x = np.concatenate([np.asarray(z_i), np.asarray(z_j)], axis=0)
    assert x.shape == (N, D) and x.dtype == np.float32
    xn = x / np.maximum(np.sqrt((x.astype(np.float64) ** 2).sum(axis=1,
                        keepdims=True)), 1e-8)
    x8 = (xn * F8SCALE).astype(F8NP)             # [8192, 1024] fp8
    x8t = np.ascontiguousarray(x8.T)             # [1024, 8192] fp8
    masks = _make_masks()

    in_maps = []
    for c in range(NCORES):
        b = ((c + 4) % NCORES) * RPC
        if c < 4:
            g2cols = b + np.arange(NG2)
        else:
            g2cols = b + (np.arange(NG2) + 512) % NG2
        cols = np.concatenate([(c * RPC + np.arange(NREG)) % N, g2cols])
        # chunk-major staging: [NCH, 128, KT, CHUNK], contiguous per
        # (chunk, partition) so each chunk DMA is one 4KB run/partition
        xt_c = x8t[:, cols].reshape(KT, 128, NCH, CHUNK)
        xall_c = np.ascontiguousarray(xt_c.transpose(2, 1, 0, 3))
        in_maps.append({"xall8": xall_c, "masks": masks})

    res = run_bass_kernel_spmd(
        nc, in_maps, core_ids=list(range(NCORES)), trace=_trace,
    )
    LAST_RESULTS = res

    # Host epilogue (O(N) numpy, f64): combine row partials with the
    # symmetric colsum partials, then mean(log denom - log E_target).
    denom = np.zeros(N, dtype=np.float64)
    ediag = np.zeros(N, dtype=np.float64)
    etarg = np.zeros(N, dtype=np.float64)
    pm = (128 * np.arange(MT)[None, :] + np.arange(128)[:, None]).ravel()
    for c in range(NCORES):
        r = res.results[c]
        rows = RPC * c + pm
        denom[rows] += r["esum"].astype(np.float64).sum(axis=2).ravel()
        ediag[rows] = r["ediag"].astype(np.float64).ravel()
        etarg[rows] = r["etarg"].astype(np.float64).ravel()
        cs = r["colsums"].astype(np.float64).sum(axis=0)
        s = NCS * CHUNK
        gcols = (RPC * c + RPC + np.arange(NCS * CHUNK)) % N
        denom[gcols] += cs[0:s]
        b = ((c + 4) % NCORES) * RPC
        if c < 4:
            g2cols = b + np.arange(NG2)
        else:
            g2cols = b + (np.arange(NG2) + 512) % NG2
        denom[g2cols[0:CHUNK]] += cs[s:s + CHUNK]
        denom[g2cols[CHUNK:]] += cs[s + CHUNK:s + NG2]
        denom[c * RPC + CHUNK + np.arange(CHUNK)] += cs[s + NG2:]
    loss = np.mean(np.log(denom - ediag) - np.log(etarg))
    return np.float32(loss)


# revision 10
# speedup vs baseline: 2.4734x; 1.0308x over previous
"""NT-Xent loss on 8 Trainium2 NeuronCores (Bass/Tile), v5.

Reference computation (B=4096, D=1024, T=0.5):
    x  = concat(z_i, z_j)                      # [8192, 1024] f32
    xn = x / ||x||                             # row-normalize
    sim = xn @ xn.T                            # [8192, 8192]
    logits = sim / T, diag masked to -inf
    loss = -mean(log_softmax(logits)[i, target(i)]), target(i) = i ^ 1

E = exp(sim/T) is symmetric, so only half the matrix need be computed.
Core c owns rows [1024c, 1024(c+1)) and sweeps ~4.25 of the 8 column
blocks:
  - blocks c+1..c+3 (mod 8) in full,
  - its own diagonal block minus the lower-left quadrant (the triangle
    trick: quadrant D10 is recovered from a column-sum of D01
    restricted to the top m-half),
  - half of block c+4, quadrant-split between the two endpoint cores
    (cores 0-3 take the diagonal quadrants Q00/Q11 of their d=4 block,
    cores 4-7 the anti-diagonal Q01/Q10)
-- together an exact single cover.  Row sums come from the ACT exp
accumulator; transposed halves are covered by DVE column-sum
accumulators shipped to the host.  The host combines partials,
subtracts the diagonal, and takes mean(log denom - log E_target) in
f64 -- an O(N) numpy epilogue.

Normalization happens on the host (O(N*D) staging prep, like the
transpose + fp8 casts); both matmul sides are fp8 at scale 16, sliced
from the same chunk tiles (the d=0 chunks ARE the own rows), so the
device is a pure DoubleRow-fp8 sweep (PE) + one wide exp per
(phase, m-tile) out of a multi-bank PSUM tile (ACT) + bf16 colsum adds
and diag/target extraction (DVE).

v4 lessons (trace): per-chunk tiles + queue-ordered DMA moved the
first matmul from 29 us to 8 us, and the HAM warm-up matmuls held the
PE at 2.4 GHz -- but the separate weight-tile DMAs ahead of chunk 1
starved phase B (3.1 us stall + a HAM cold dip), and the small-matmul
phase E stalled 1.8 us behind the 2 us activations of big phase D.
v5 slices weights from the chunk tiles (1 MB less DMA, chunk 1 lands
in time), runs the phases in chunk-count-increasing order
A(c0) B(c1) E(c8/9) C(c2-3) D(c4-7) so activation width only grows,
and drops the 16 triangle-trick matmuls.
"""

import numpy as np
import ml_dtypes
from contextlib import ExitStack

import concourse.bass as bass
import concourse.tile as tile
from concourse import bacc, mybir
from concourse.bass_utils import run_bass_kernel_spmd

F32 = mybir.dt.float32
BF16 = mybir.dt.bfloat16
F8 = mybir.dt.float8e4
BF = ml_dtypes.bfloat16
F8NP = ml_dtypes.float8_e4m3
F8SCALE = 16.0
EXPSCALE = 2.0 / (F8SCALE * F8SCALE)   # exp((16 xn_r . 16 xn_c) / (256 T))

B = 4096
D = 1024
N = 2 * B            # 8192 rows total
NCORES = 8
RPC = N // NCORES    # 1024 rows per core
KT = D // 128        # 8 contraction partition-tiles
MT = RPC // 128      # 8 row tiles per core
CHUNK = 512
NREG = 4 * RPC       # regular swept columns (blocks d=0..3)
NG2 = RPC            # staged d=4 columns (chunk 8 for m<4, chunk 9 for m>=4)
NCOL = NREG + NG2    # 5120 staged columns
NCH = NCOL // CHUNK  # 10 column chunks
NCS = 6              # chunks whose colsums ship (d=1..3)
NWARM = 4            # dummy PE warm-up matmuls during the DMA prologue

# phases: (chunk list, m range); -1 means chunk 8 or 9 by m-half.
# Phase A covers only the top m-half of chunk 0 (triangle trick).
PHASES = [
    ([0], range(0, 4)),
    ([1], range(MT)),
    ([-1], range(MT)),
    ([2, 3], range(MT)),
    ([4, 5], range(MT)),
    ([6, 7], range(MT)),
]
NPH = len(PHASES)

_NC_CACHE = {}
LAST_RESULTS = None  # BassKernelResults of the most recent run (for test.py)


def _build_program():
    nc = bacc.Bacc("TRN2", target_bir_lowering=False, debug=False)

    xall8 = nc.dram_tensor("xall8", [NCH, 128, KT, CHUNK], F8, kind="ExternalInput")
    masks = nc.dram_tensor("masks", [128, 256], BF16, kind="ExternalInput")
    esum_out = nc.dram_tensor("esum", [128, MT, NPH], F32, kind="ExternalOutput")
    ediag_out = nc.dram_tensor("ediag", [128, MT], F32, kind="ExternalOutput")
    etarg_out = nc.dram_tensor("etarg", [128, MT], F32, kind="ExternalOutput")
    # [cs13 d=1..3 | cs4a | cs4b | cs_d01]
    cs_out = nc.dram_tensor("colsums", [128, NCS * CHUNK + NG2 + CHUNK], BF16,
                            kind="ExternalOutput")

    ADD = mybir.AluOpType.add
    EXP = mybir.ActivationFunctionType.Exp

    with tile.TileContext(nc) as tc, ExitStack() as ctx:
        consts = ctx.enter_context(tc.tile_pool(name="consts", bufs=1))
        xin_pool = ctx.enter_context(tc.tile_pool(name="xin", bufs=1))
        exp_pool = ctx.enter_context(tc.tile_pool(name="exp", bufs=3))
        scr_pool = ctx.enter_context(tc.tile_pool(name="scr", bufs=2))
        stat_pool = ctx.enter_context(tc.tile_pool(name="stat", bufs=1))
        ps_pool = ctx.enter_context(tc.tile_pool(name="ps", bufs=2, space="PSUM"))
        # 2 x 4-bank PSUM tiles == all 8 banks

        # PE warm-up operand: memset'd zeros, no DMA dependency
        zbf = consts.tile([128, 640], BF16)
        nc.vector.memset(zbf[:], 0.0)

        mask_sb = consts.tile([128, 256], BF16)
        xc = [xin_pool.tile([128, KT, CHUNK], F8, name=f"xc{j}")
              for j in range(NCH)]

        # DMA issue order by first use: chunk 0 quartered across four
        # queues, chunk 1 halved, then chunks 8/9 (phase E is third),
        # then the rest round-robin.
        nc.scalar.dma_start(xc[0][:, 0:3], xall8[0, :, 0:3])
        nc.sync.dma_start(xc[0][:, 3:6], xall8[0, :, 3:6])
        nc.gpsimd.dma_start(xc[0][:, 6:KT], xall8[0, :, 6:KT])
        h = KT // 2
        nc.scalar.dma_start(xc[1][:, 0:h], xall8[1, :, 0:h])
        nc.sync.dma_start(xc[1][:, h:KT], xall8[1, :, h:KT])
        nc.gpsimd.dma_start(mask_sb[:], masks[:])
        nc.gpsimd.dma_start(xc[8][:], xall8[8])
        nc.scalar.dma_start(xc[9][:], xall8[9])
        nc.sync.dma_start(xc[2][:], xall8[2])
        nc.gpsimd.dma_start(xc[3][:], xall8[3])
        nc.scalar.dma_start(xc[4][:], xall8[4])
        nc.sync.dma_start(xc[5][:], xall8[5])
        nc.gpsimd.dma_start(xc[6][:], xall8[6])
        nc.scalar.dma_start(xc[7][:], xall8[7])

        esum = stat_pool.tile([128, MT, NPH], F32)
        ediag = stat_pool.tile([128, MT], F32)
        etarg = stat_pool.tile([128, MT], F32)
        cs13 = stat_pool.tile([128, NCS * CHUNK], BF16)
        cs4a = stat_pool.tile([128, CHUNK], BF16)
        cs4b = stat_pool.tile([128, CHUNK], BF16)
        csd01 = stat_pool.tile([128, CHUNK], BF16)
        nc.vector.memset(esum[:], 0.0)
        nc.vector.memset(cs13[:], 0.0)
        nc.vector.memset(cs4a[:], 0.0)
        nc.vector.memset(cs4b[:], 0.0)
        nc.vector.memset(csd01[:], 0.0)

        # HAM warm-up: dummy bf16 matmuls fill the otherwise-idle PE
        # during the first chunk's DMA so the 2.4 GHz clock-gate opens
        # before the real sweep starts.
        ps_w = ps_pool.tile([128, 2048], F32, name="ps", tag="ps")
        for _ in range(NWARM):
            nc.tensor.matmul(ps_w[:, 0:CHUNK], lhsT=zbf[:, 0:128],
                             rhs=zbf[:, 128:640], start=True, stop=True)

        for ph, (chunks, ms) in enumerate(PHASES):
            for m in ms:
                cj = [(8 if m < 4 else 9) if j < 0 else j for j in chunks]
                w = CHUNK * len(cj)
                ps = ps_pool.tile([128, 2048], F32, name="ps", tag="ps")
                for t in range(KT // 2):
                    for ci, j in enumerate(cj):
                        nc.tensor.matmul(
                            ps[:, CHUNK * ci:CHUNK * (ci + 1)],
                            lhsT=xc[m // 4][:, 2 * t:2 * t + 2,
                                            128 * (m % 4):128 * (m % 4) + 128],
                            rhs=xc[j][:, 2 * t:2 * t + 2, :],
                            start=(t == 0), stop=(t == KT // 2 - 1),
                            perf_mode=mybir.MatmulPerfMode.DoubleRow,
                        )
                esb = exp_pool.tile([128, w], BF16)
                nc.scalar.activation(
                    esb[:], ps[:, 0:w], EXP, scale=EXPSCALE,
                    accum_out=esum[:, m, ph:ph + 1],
                )
                if (ph == 0 and m < 4) or (ph == 1 and m >= 4):
                    off = 128 * m - (0 if ph == 0 else CHUNK)
                    scr = scr_pool.tile([128, 128], BF16)
                    nc.vector.tensor_mul(
                        scr[:], esb[:, off:off + 128], mask_sb[:, 0:128])
                    nc.vector.tensor_reduce(
                        ediag[:, m:m + 1], scr[:],
                        axis=mybir.AxisListType.X, op=ADD)
                    scr2 = scr_pool.tile([128, 128], BF16)
                    nc.vector.tensor_mul(
                        scr2[:], esb[:, off:off + 128], mask_sb[:, 128:256])
                    nc.vector.tensor_reduce(
                        etarg[:, m:m + 1], scr2[:],
                        axis=mybir.AxisListType.X, op=ADD)
                if ph == 1 and m < 4:
                    nc.vector.tensor_add(csd01[:], csd01[:], esb[:])
                elif ph == 2:
                    tgt = cs4a if m < 4 else cs4b
                    nc.vector.tensor_add(tgt[:], tgt[:], esb[:])
                elif ph == 3:
                    nc.vector.tensor_add(cs13[:, 0:1024], cs13[:, 0:1024],
                                         esb[:])
                elif ph == 4:
                    nc.vector.tensor_add(cs13[:, 1024:2048],
                                         cs13[:, 1024:2048], esb[:])
                elif ph == 5:
                    nc.vector.tensor_add(cs13[:, 2048:3072],
                                         cs13[:, 2048:3072], esb[:])
            # ship each accumulator as soon as its last add retires so
            # only the final phase's piece remains for the tail
            s = NCS * CHUNK
            if ph == 1:
                nc.gpsimd.dma_start(cs_out[:, s + 2 * CHUNK:], csd01[:])
                nc.gpsimd.dma_start(ediag_out[:], ediag[:])
                nc.gpsimd.dma_start(etarg_out[:], etarg[:])
            elif ph == 2:
                nc.gpsimd.dma_start(cs_out[:, s:s + CHUNK], cs4a[:])
                nc.gpsimd.dma_start(cs_out[:, s + CHUNK:s + 2 * CHUNK],
                                    cs4b[:])
            elif ph == 3:
                nc.gpsimd.dma_start(cs_out[:, 0:1024], cs13[:, 0:1024])
            elif ph == 4:
                nc.gpsimd.dma_start(cs_out[:, 1024:2048], cs13[:, 1024:2048])

        nc.sync.dma_start(cs_out[:, 2048:2560], cs13[:, 2048:2560])
        nc.gpsimd.dma_start(cs_out[:, 2560:3072], cs13[:, 2560:3072])
        nc.sync.dma_start(esum_out[:], esum[:])

    nc.finalize()
    return nc


def _get_program():
    if "nc" not in _NC_CACHE:
        _NC_CACHE["nc"] = _build_program()
    return _NC_CACHE["nc"]


def _make_masks():
    m = np.zeros((128, 256), dtype=np.float32)
    p = np.arange(128)
    m[p, p] = 1.0              # identity: diagonal extraction
    m[p, 128 + (p ^ 1)] = 1.0  # pair-swap: target extraction
    return m.astype(BF)


def kernel(z_i: np.ndarray, z_j: np.ndarray, _trace: bool = False) -> np.ndarray:
    global LAST_RESULTS
    nc = _get_program()

    x = np.concatenate([np.asarray(z_i), np.asarray(z_j)], axis=0)
    assert x.shape == (N, D) and x.dtype == np.float32
    xn = x / np.maximum(np.sqrt((x.astype(np.float64) ** 2).sum(axis=1,
                        keepdims=True)), 1e-8)
    x8 = (xn * F8SCALE).astype(F8NP)             # [8192, 1024] fp8
    x8t = np.ascontiguousarray(x8.T)             # [1024, 8192] fp8
    masks = _make_masks()

    in_maps = []
    for c in range(NCORES):
        b = ((c + 4) % NCORES) * RPC
        if c < 4:
            g2cols = b + np.arange(NG2)
        else:
            g2cols = b + (np.arange(NG2) + 512) % NG2
        cols = np.concatenate([(c * RPC + np.arange(NREG)) % N, g2cols])
        # chunk-major staging: [NCH, 128, KT, CHUNK], contiguous per
        # (chunk, partition) so each chunk DMA is one 4KB run/partition
        xt_c = x8t[:, cols].reshape(KT, 128, NCH, CHUNK)
        xall_c = np.ascontiguousarray(xt_c.transpose(2, 1, 0, 3))
        in_maps.append({"xall8": xall_c, "masks": masks})

    res = run_bass_kernel_spmd(
        nc, in_maps, core_ids=list(range(NCORES)), trace=_trace,
    )
    LAST_RESULTS = res

    # Host epilogue (O(N) numpy, f64): combine row partials with the
    # symmetric colsum partials, then mean(log denom - log E_target).
    denom = np.zeros(N, dtype=np.float64)
    ediag = np.zeros(N, dtype=np.float64)
    etarg = np.zeros(N, dtype=np.float64)
    pm = (128 * np.arange(MT)[None, :] + np.arange(128)[:, None]).ravel()
    for c in range(NCORES):
        r = res.results[c]
        rows = RPC * c + pm
        denom[rows] += r["esum"].astype(np.float64).sum(axis=2).ravel()
        ediag[rows] = r["ediag"].astype(np.float64).ravel()
        etarg[rows] = r["etarg"].astype(np.float64).ravel()
        cs = r["colsums"].astype(np.float64).sum(axis=0)
        s = NCS * CHUNK
        gcols = (RPC * c + RPC + np.arange(NCS * CHUNK)) % N
        denom[gcols] += cs[0:s]
        b = ((c + 4) % NCORES) * RPC
        if c < 4:
            g2cols = b + np.arange(NG2)
        else:
            g2cols = b + (np.arange(NG2) + 512) % NG2
        denom[g2cols[0:CHUNK]] += cs[s:s + CHUNK]
        denom[g2cols[CHUNK:]] += cs[s + CHUNK:s + NG2]
        denom[c * RPC + CHUNK + np.arange(CHUNK)] += cs[s + NG2:]
    loss = np.mean(np.log(denom - ediag) - np.log(etarg))
    return np.float32(loss)
